# revision 7
# baseline (speedup 1.0000x reference)
"""Trainium2 Bass kernel for nn_AdaptiveSample (sparse adaptive 5x5 sampling).

Computes, for full inputs
    depth [2,1,256,512] f32, features [2,32,256,512] f32,
    guide_weight [2,256,512,25] f32, sample_idx [15] int32:
    out[b,c,y,x] = sum_s softmax_s(valid*pos_w*guide)_s * features[b,c,y+dy_s-2,x+dx_s-2]
returning (out, features) exactly like the reference nn.Module.

Strategy: shard H=256 over 8 NeuronCores (32 rows each, 2-row halos resolved on
host while slicing shards).  Per core:
  - features live in SBUF as bf16 "slabs" [128=(xquad,chan), (b, y+halo, x+halo)]
    so every 5x5 offset is a pure free-dim address shift (two x-shifted copies
    keep bf16 4-byte alignment for odd dx).
  - softmax pipeline (validity, guide gather, exp, normalize) runs in a compact
    pixel-partition layout [128 lanes, D*256].
  - normalized weights are broadcast across the 32 channel partitions with tiny
    k=4 bf16 matmuls on the otherwise-idle TensorEngine (selector stationary),
    evacuated PSUM->SBUF as bf16 by the ScalarEngine.
  - VectorEngine does the D-offset multiply-accumulate in bf16 (2x DVE mode).
Duplicate sample offsets are merged (weight folded into the exp bias), so the
MAC runs over D distinct offsets instead of 15.
"""
import os
import sys

for _p in ("/opt/trn_rl_repo",):
    if os.path.isdir(_p) and _p not in sys.path:
        sys.path.append(_p)

import numpy as np
import ml_dtypes

from concourse import bass, mybir
from concourse import tile
from concourse.bass_utils import run_bass_kernel_spmd

BF16 = ml_dtypes.bfloat16
F32 = np.float32

B, C, H, W = 2, 32, 256, 512
KS, PAD, DMAX = 5, 2, 192.0
NCORES = 8
HS = H // NCORES          # 32 rows per core
NQ = 4                    # x quadrants of 128
XL = W // NQ              # 128
SLAB_Y = HS + 2 * PAD     # 36
SLAB_X = XL + 2 * PAD     # 132
NLANE_T = 256             # pixel-plane free size per lane
NPIX = B * HS * W         # 32768 pixels per core
QUARTER = 2048            # free elems per MAC quarter (16 y rows)

_graph_cache = {}


def _build_graph(D, dyv, dxv, pos_d, counts):
    """Build the SPMD Bass graph for one core (identical across cores)."""
    nc = bass.Bass(trn_type="TRN2", debug=False, enable_partition_id=False)
    dt_bf = mybir.dt.bfloat16
    dt_f32 = mybir.dt.float32

    feat_a = nc.declare_dram_parameter("feat_a", [128, B, SLAB_Y, SLAB_X], dt_bf, isOutput=False)
    feat_b = nc.declare_dram_parameter("feat_b", [128, B, SLAB_Y, SLAB_X], dt_bf, isOutput=False)
    guide = nc.declare_dram_parameter("guide", [128, D, NLANE_T], dt_f32, isOutput=False)
    depthp = nc.declare_dram_parameter("depthp", [128, D, NLANE_T], dt_f32, isOutput=False)
    sel = nc.declare_dram_parameter("sel", [NQ, 128], dt_bf, isOutput=False)
    out_ext = nc.declare_dram_parameter("out", [128, B, HS, XL], dt_f32, isOutput=True)

    MULT = mybir.AluOpType.mult
    ADD = mybir.AluOpType.add
    IS_GT = mybir.AluOpType.is_gt
    IS_LT = mybir.AluOpType.is_lt
    EXP = mybir.ActivationFunctionType.Exp
    COPY = mybir.ActivationFunctionType.Copy

    with tile.TileContext(nc) as tc:
        with (
            tc.tile_pool(name="big", bufs=1) as big,
            tc.tile_pool(name="pipe", bufs=1) as pipe,
            tc.tile_pool(name="xrl", bufs=2) as xrl,
            tc.tile_pool(name="wsb", bufs=3) as wsb,
            tc.tile_pool(name="tmp", bufs=3) as tmpp,
            tc.tile_pool(name="psum", bufs=2, space="PSUM") as psp,
        ):
            fa = big.tile([128, B, SLAB_Y, SLAB_X], dt_bf, tag="fa")
            fb = big.tile([128, B, SLAB_Y, SLAB_X], dt_bf, tag="fb")
            acc = big.tile([128, B, HS, XL], dt_bf, tag="acc")
            S = big.tile([NQ, 128], dt_bf, tag="sel")
            nc.sync.dma_start(out=fa[:, :, :, :], in_=feat_a[:, :, :, :])
            nc.sync.dma_start(out=fb[:, :, :, :], in_=feat_b[:, :, :, :])
            nc.sync.dma_start(out=S[:, :], in_=sel[:, :])

            g = pipe.tile([128, D, NLANE_T], dt_f32, tag="g")
            dp = pipe.tile([128, D, NLANE_T], dt_f32, tag="dp")
            nc.sync.dma_start(out=g[:, :, :], in_=guide[:, :, :])
            nc.sync.dma_start(out=dp[:, :, :], in_=depthp[:, :, :])

            # validity: v = (dp > 0) * (dp < DMAX)
            t1 = pipe.tile([128, D, NLANE_T], dt_f32, tag="t1")
            v = pipe.tile([128, D, NLANE_T], dt_f32, tag="v")
            nc.vector.tensor_scalar(t1[:, :, :], dp[:, :, :], 0.0, None, IS_GT)
            nc.vector.scalar_tensor_tensor(v[:, :, :], dp[:, :, :], DMAX, t1[:, :, :], IS_LT, MULT)
            # u = v * guide  (pre-softmax argument without pos_w)
            u = pipe.tile([128, D, NLANE_T], dt_f32, tag="u")
            nc.vector.tensor_tensor(u[:, :, :], v[:, :, :], g[:, :, :], MULT)
            # e_d = exp(pos_w_d * u_d + ln(count_d)); bias passed as [128,1] AP
            bias_vals = sorted({float(np.log(cnt)) for cnt in counts})
            bias_tiles = {}
            for bv in bias_vals:
                bt = pipe.tile([128, 1], dt_f32, tag=f"bias{bv:.4f}")
                nc.vector.memset(bt[:, :], bv)
                bias_tiles[bv] = bt
            e = pipe.tile([128, D, NLANE_T], dt_f32, tag="e")
            for d in range(D):
                bv = float(np.log(counts[d]))
                nc.scalar.activation(e[:, d, :], u[:, d, :], EXP,
                                     bias=bias_tiles[bv][:, :], scale=float(pos_d[d]))
            # den = sum_d e_d ; r = 1/den
            den = pipe.tile([128, NLANE_T], dt_f32, tag="den")
            nc.vector.tensor_tensor(den[:, :], e[:, 0, :], e[:, 1, :], ADD)
            for d in range(2, D):
                nc.vector.tensor_tensor(den[:, :], den[:, :], e[:, d, :], ADD)
            r = pipe.tile([128, NLANE_T], dt_f32, tag="r")
            nc.vector.reciprocal(r[:, :], den[:, :])
            # wt_d = e_d * r  (bf16)
            wt = pipe.tile([128, D, NLANE_T], dt_bf, tag="wt")
            for d in range(D):
                nc.vector.tensor_tensor(wt[:, d, :], e[:, d, :], r[:, :], MULT)
            # bounce the weight planes through DRAM to re-lay them out as the
            # broadcast-matmul moving operand (partition-crossing SBUF->SBUF
            # DMA with rearranged APs is unsound in this stack)
            wtb = nc.dram_tensor("wtb", [128, D, NLANE_T], dt_bf)
            nc.sync.dma_start(out=wtb[:, :, :], in_=wt[:, :, :])
            wtb_v = wtb.ap().rearrange(
                "(by xh) d (xx xl) -> by xh d xx xl", by=B * HS, xh=2, xx=2, xl=XL)

            # per-offset: relayout -> broadcast matmul -> bf16 evac -> MAC
            for d in range(D):
                dy, dx = int(dyv[d]), int(dxv[d])
                fsrc, dxq = (fa, dx) if dx % 2 == 0 else (fb, dx - 1)
                # X rows ordered q = xx*2 + xh (host slab layout matches)
                X = xrl.tile([NQ, B * HS, XL], dt_bf, tag="X")
                for xx in range(2):
                    nc.sync.dma_start(
                        out=X[xx * 2:(xx + 1) * 2, :, :],
                        in_=wtb_v[:, :, d, xx, :].transpose([1, 0, 2]),
                    )
                for j in range(4):          # quarter = (b, yhalf)
                    bq, yh = j // 2, (j % 2) * 16
                    by0 = bq * HS + yh
                    ps = psp.tile([128, QUARTER], dt_f32, tag="ps")
                    for ci in range(4):     # 512-col matmul chunks (4 y rows each)
                        nc.tensor.matmul(
                            ps[:, ci * 512:(ci + 1) * 512],
                            lhsT=S[:, :],
                            rhs=X[:, by0 + 4 * ci: by0 + 4 * ci + 4, :],
                            start=True, stop=True,
                        )
                    w_sb = wsb.tile([128, 16, XL], dt_bf, tag="w_sb")
                    nc.scalar.activation(w_sb[:, :, :],
                                         ps[:, :].rearrange("p (y xl) -> p y xl", y=16, xl=XL),
                                         COPY)
                    fsl = fsrc[:, bq, yh + dy: yh + dy + 16, dxq: dxq + XL]
                    if d == 0:
                        nc.vector.tensor_tensor(acc[:, bq, yh:yh + 16, :], fsl, w_sb[:, :, :], MULT)
                    else:
                        t = tmpp.tile([128, 16, XL], dt_bf, tag="t")
                        nc.vector.tensor_tensor(t[:, :, :], fsl, w_sb[:, :, :], MULT)
                        nc.vector.tensor_tensor(acc[:, bq, yh:yh + 16, :],
                                                acc[:, bq, yh:yh + 16, :], t[:, :, :], ADD)

            # cast bf16 -> f32 on the way out (SWDGE cast DMA)
            nc.gpsimd.dma_start(out=out_ext[:, :, :, :], in_=acc[:, :, :, :])

    _split_excess_waits(nc)
    return nc


def _split_excess_waits(nc, max_waits=1):
    """walrus in this container rejects >1 chained sync-wait per instruction;
    spill extras onto preceding sequencer NOPs."""
    n = 0
    for fn in nc.m.functions:
        for bb in fn.blocks:
            new = []
            for inst in bb.instructions:
                si = inst.sync_info
                w = list(si.on_wait) if si is not None else []
                if len(w) > max_waits:
                    excess = w[max_waits:]
                    si.on_wait = w[:max_waits]
                    for i in range(0, len(excess), max_waits):
                        nop = mybir.InstNoOp(name=nc.get_next_instruction_name(), ins=[], outs=[])
                        nop.engine = inst.engine
                        nsi = nop.sync_info
                        if nsi is None:
                            nop.sync_info = mybir.SyncInfo(on_wait=excess[i:i + max_waits], on_update=[])
                        else:
                            nsi.on_wait = excess[i:i + max_waits]
                        nc.register_instruction(nop)
                        new.append(nop)
                        n += 1
                new.append(inst)
            bb.instructions = new
    return n


def _prep_inputs(depth, features, guide_weight, sample_idx):
    """Shard + lay out the full inputs for the 8 cores. Returns in_maps, meta."""
    si = np.asarray(sample_idx).astype(np.int64)
    vals, counts = np.unique(si, return_counts=True)
    D = len(vals)
    ctr = KS // 2
    px = (si % KS).astype(np.float64)
    py = (si // KS).astype(np.float64)
    Z = np.exp(-0.5 * np.sqrt((px - ctr) ** 2 + (py - ctr) ** 2)).sum()
    pos_d = np.exp(-0.5 * np.sqrt(((vals % KS) - ctr) ** 2 + ((vals // KS) - ctr) ** 2)) / Z
    dyv = (vals // KS).astype(int)          # 0..4 offsets in padded coords
    dxv = (vals % KS).astype(int)

    feats_bf = features.astype(BF16)
    # padded feature planes, then per-core slabs [128=(q,c), B, SLAB_Y, SLAB_X]
    fpad = np.zeros((B, C, H + 2 * PAD, W + 2 * PAD + 1), BF16)
    fpad[:, :, PAD:PAD + H, PAD:PAD + W] = feats_bf
    dpad = np.zeros((B, H + 2 * PAD, W + 2 * PAD), F32)
    dpad[:, PAD:PAD + H, PAD:PAD + W] = depth.reshape(B, H, W)

    in_maps = []
    sel = np.zeros((NQ, 128), BF16)
    for q in range(NQ):
        sel[q, q * C:(q + 1) * C] = 1.0
    for core in range(NCORES):
        r0 = core * HS
        # feature slabs: x window for quadrant q: [q*XL, q*XL+SLAB_X) in padded coords
        fa = np.empty((NQ, C, B, SLAB_Y, SLAB_X), BF16)
        fb = np.empty((NQ, C, B, SLAB_Y, SLAB_X), BF16)
        for q in range(NQ):
            # partition-group q covers global x window (q%2)*256 + (q//2)*128
            w0 = (q % 2) * 256 + (q // 2) * XL
            blk_a = fpad[:, :, r0:r0 + SLAB_Y, w0:w0 + SLAB_X]
            blk_b = fpad[:, :, r0:r0 + SLAB_Y, w0 + 1:w0 + SLAB_X + 1]
            fa[q] = np.transpose(blk_a, (1, 0, 2, 3))
            fb[q] = np.transpose(blk_b, (1, 0, 2, 3))
        fa = fa.reshape(128, B, SLAB_Y, SLAB_X)
        fb = fb.reshape(128, B, SLAB_Y, SLAB_X)

        # pixel-plane tensors [128 lanes, D, 256]; lane = (b*HS+y)*2+xh
        gsel = guide_weight[:, r0:r0 + HS, :, :][..., vals]            # B,HS,W,D
        gsel = np.ascontiguousarray(
            np.transpose(gsel.reshape(B, HS, 2, NLANE_T, D), (0, 1, 2, 4, 3))
        ).reshape(128, D, NLANE_T).astype(F32)
        dsh = np.empty((B, HS, 2, D, NLANE_T), F32)
        for d in range(D):
            blk = dpad[:, r0 + dyv[d]:r0 + dyv[d] + HS, dxv[d]:dxv[d] + W]   # B,HS,W
            dsh[:, :, :, d, :] = blk.reshape(B, HS, 2, NLANE_T)
        dsh = dsh.reshape(128, D, NLANE_T)

        in_maps.append({
            "feat_a": fa, "feat_b": fb, "guide": gsel, "depthp": dsh, "sel": sel,
        })
    return in_maps, (D, dyv, dxv, pos_d, counts)


def kernel(depth, features, guide_weight, sample_idx):
    depth = np.asarray(depth)
    features = np.asarray(features)
    guide_weight = np.asarray(guide_weight)
    sample_idx = np.asarray(sample_idx)

    in_maps, meta = _prep_inputs(depth, features, guide_weight, sample_idx)
    D, dyv, dxv, pos_d, counts = meta

    key = (tuple(dyv), tuple(dxv), tuple(np.round(pos_d, 10)), tuple(counts))
    nc = _graph_cache.get(key)
    if nc is None:
        nc = _build_graph(D, dyv, dxv, pos_d, counts)
        _graph_cache[key] = nc

    res = run_bass_kernel_spmd(nc, in_maps, core_ids=list(range(NCORES)))

    out = np.empty((B, C, H, W), F32)
    for core in range(NCORES):
        r0 = core * HS
        o = res.results[core]["out"].reshape(NQ, C, B, HS, XL)
        for q in range(NQ):
            w0 = (q % 2) * 256 + (q // 2) * XL
            out[:, :, r0:r0 + HS, w0:w0 + XL] = np.transpose(o[q], (1, 0, 2, 3))
    return out, features
